# revision 26
# baseline (speedup 1.0000x reference)
"""MoE (Mixtral sparse block) Trainium2 kernel.

Strategy (expert-parallel, per sharding hint):
  - Host: compute router logits/softmax/top-2 in f32 (tiny: T x E x H),
    dispatch tokens to experts (this is the "all-to-all" -- done during
    host-side sharding, which the full-IO contract allows).
  - Device (8 cores, SPMD): core e runs expert e's FFN on its gathered
    tokens: y = (silu(x @ w1^T) * (x @ w3^T)) @ w2^T. bf16 operands,
    f32 PSUM accumulation, f32 output.
  - Host: combine = scatter-add rw-weighted expert outputs; return
    (final, router_logits) exactly like the reference.

Layouts: all device tensors are pre-transposed on host so the expert
weights are the stationary matmul operands and activations stream as the
moving operand; both FFN stages consume/produce [feature, token] layout,
so no on-device transposes are needed.
"""

import os
import numpy as np
import ml_dtypes

H = 1024
F = 2048
E = 8
TOPK = 2
P = 128
KH = H // P   # 8  contraction subtiles for H
KF = F // P   # 16 contraction subtiles for F
NBLK = 512    # moving-dim (token) block per matmul

_cache = {}


def _build(C):
    """Build the SPMD Bass program for per-core token capacity C."""
    import concourse.mybir as mybir
    from concourse import bacc
    from concourse.tile import TileContext

    dt = mybir.dt
    nc = bacc.Bacc()

    x_d = nc.declare_dram_parameter("xt", [P, KH, C], dt.bfloat16, isOutput=False)
    w1_d = nc.declare_dram_parameter("w1t", [P, KH, F], dt.bfloat16, isOutput=False)
    w3_d = nc.declare_dram_parameter("w3t", [P, KH, F], dt.bfloat16, isOutput=False)
    w2_d = nc.declare_dram_parameter("w2t", [P, KF, H], dt.bfloat16, isOutput=False)
    y_d = nc.declare_dram_parameter("yt", [P, KH, C], dt.float32, isOutput=True)

    cblocks = [(c0, min(NBLK, C - c0)) for c0 in range(0, C, NBLK)]
    NCB = len(cblocks)
    # w1/w3 DMA group sizes (in 128-col f tiles): small first so the first
    # matmuls wait on ~0.5MB, not 2MB.
    FGS = [1, 1, 2, 4, 4, 4]
    assert sum(FGS) == KF
    FG0 = [sum(FGS[:i]) for i in range(len(FGS))]  # start f-tile of group
    NFG = len(FGS)

    def fgroup(f):
        for g in range(NFG):
            if FG0[g] <= f < FG0[g] + FGS[g]:
                return g, f - FG0[g]
        raise AssertionError

    with TileContext(nc) as tc:
        with (
            tc.tile_pool(name="const", bufs=1) as cpool,
            tc.tile_pool(name="gated", bufs=2) as gpool,
            tc.tile_pool(name="tmp", bufs=3) as tpool,
            tc.tile_pool(name="outp", bufs=3) as opool,
            tc.tile_pool(name="psum", bufs=2, space="PSUM") as ppool,
        ):
            # PE warm-up: dummy matmuls on a zeroed tile from t=0 keep the
            # HAM activity window busy so the clock gate opens (1.2->2.4GHz)
            # before the first real matmuls are ready.
            zt = cpool.tile([P, NBLK], dt.bfloat16, tag="warm", name="warm")
            nc.vector.memset(zt[:], 0.0)
            pw = ppool.tile([P, NBLK], dt.float32, tag="wps", name="wps")
            for i in range(16):
                nc.tensor.matmul(pw[:], lhsT=zt[:, :P], rhs=zt[:],
                                 start=(i == 0), stop=(i == 15))

            # Split SBUF residents into small tiles so the first matmuls
            # only depend on the first ~1MB of DMA, not the full 15MB.
            xcb = [cpool.tile([P, KH, cb], dt.bfloat16, tag=f"x{j}", name=f"x{j}")
                   for j, (c0, cb) in enumerate(cblocks)]
            w1g = [cpool.tile([P, KH, FGS[g] * P], dt.bfloat16, tag=f"w1g{g}", name=f"w1g{g}")
                   for g in range(NFG)]
            w3g = [cpool.tile([P, KH, FGS[g] * P], dt.bfloat16, tag=f"w3g{g}", name=f"w3g{g}")
                   for g in range(NFG)]
            w2g = [cpool.tile([P, KF, P], dt.bfloat16, tag=f"w2g{h}", name=f"w2g{h}")
                   for h in range(KH)]

            # DMA in rough need-order.
            nc.sync.dma_start(xcb[0][:], x_d[:, :, cblocks[0][0]:cblocks[0][0] + cblocks[0][1]])
            for g in range(NFG):
                nc.sync.dma_start(w1g[g][:], w1_d[:, :, FG0[g] * P:(FG0[g] + FGS[g]) * P])
                nc.sync.dma_start(w3g[g][:], w3_d[:, :, FG0[g] * P:(FG0[g] + FGS[g]) * P])
            for j in range(1, NCB):
                c0, cb = cblocks[j]
                nc.sync.dma_start(xcb[j][:], x_d[:, :, c0:c0 + cb])
            for h in range(KH):
                nc.sync.dma_start(w2g[h][:], w2_d[:, :, h * P:(h + 1) * P])

            all_gts = {}

            def emit_s1(j, f_lo, f_hi):
                c0, cb = cblocks[j]
                gts = all_gts.setdefault(j, [gpool.tile([P, cb], dt.bfloat16, tag=f"g{f}", name=f"g{j}_{f}")
                                             for f in range(KF)])
                for f in range(f_lo, f_hi):
                    g, fi = fgroup(f)
                    ph1 = ppool.tile([P, NBLK], dt.float32, tag="h1")
                    ph3 = ppool.tile([P, NBLK], dt.float32, tag="h3")
                    for k in range(KH):
                        nc.tensor.matmul(
                            ph1[:, :cb],
                            lhsT=w1g[g][:, k, fi * P:(fi + 1) * P],
                            rhs=xcb[j][:, k, :],
                            start=(k == 0),
                            stop=(k == KH - 1),
                        )
                    for k in range(KH):
                        nc.tensor.matmul(
                            ph3[:, :cb],
                            lhsT=w3g[g][:, k, fi * P:(fi + 1) * P],
                            rhs=xcb[j][:, k, :],
                            start=(k == 0),
                            stop=(k == KH - 1),
                        )
                    st = tpool.tile([P, NBLK], dt.float32, tag="sig")
                    nc.scalar.activation(
                        st[:, :cb], ph1[:, :cb],
                        mybir.ActivationFunctionType.Sigmoid,
                    )
                    nc.vector.tensor_mul(st[:, :cb], st[:, :cb], ph1[:, :cb])
                    nc.vector.tensor_mul(gts[f][:], st[:, :cb], ph3[:, :cb])

            def emit_s3(j):
                c0, cb = cblocks[j]
                gts = all_gts[j]
                for h in range(KH):
                    po = ppool.tile([P, NBLK], dt.float32, tag="out")
                    for k2 in range(KF):
                        nc.tensor.matmul(
                            po[:, :cb],
                            lhsT=w2g[h][:, k2, :],
                            rhs=gts[k2][:],
                            start=(k2 == 0),
                            stop=(k2 == KF - 1),
                        )
                    ot = opool.tile([P, NBLK], dt.float32, tag="ot")
                    nc.vector.tensor_copy(ot[:, :cb], po[:, :cb])
                    nc.sync.dma_start(y_d[:, h, c0:c0 + cb], ot[:, :cb])

            # Interleave: before each block's stage-3, start the next
            # block's first two stage-1 f-tiles so the PE has work while
            # the DVE finishes the current block's gated tiles.
            for j in range(NCB):
                emit_s1(j, 0 if j == 0 else 2, KF)
                if j + 1 < NCB:
                    emit_s1(j + 1, 0, 2)
                emit_s3(j)
    nc.finalize()
    return nc


def _route(x, gate_w):
    """f32 router matching the reference: softmax -> top-2 -> renormalize."""
    logits = x @ gate_w.T                                   # [T, E] f32
    m = logits.max(axis=-1, keepdims=True)
    p = np.exp((logits - m).astype(np.float32))
    p /= p.sum(axis=-1, keepdims=True)
    sel = np.argsort(-p, axis=-1, kind="stable")[:, :TOPK]  # [T, k]
    rw = np.take_along_axis(p, sel, axis=-1)
    rw = (rw / rw.sum(axis=-1, keepdims=True)).astype(np.float32)
    return logits, sel, rw


def kernel(hidden_states, gate_w, w1, w2, w3):
    from concourse.bass_utils import run_bass_kernel_spmd

    b, s, h = hidden_states.shape
    x = np.ascontiguousarray(hidden_states, dtype=np.float32).reshape(-1, h)
    T = x.shape[0]
    gate_w = np.asarray(gate_w, dtype=np.float32)

    logits, sel, rw = _route(x, gate_w)

    # token lists per expert
    toks, wts = [], []
    for e in range(E):
        mask = sel == e
        te = np.nonzero(mask.any(axis=-1))[0]
        toks.append(te)
        wts.append((rw * mask).sum(axis=-1)[te].astype(np.float32))
    maxc = max(len(t) for t in toks)
    # Cap device capacity at 1024 (two clean 512 blocks, zero padding);
    # the few overflow tokens of above-average experts (~1% of the load)
    # are folded in on the host during the combine. If overflow would
    # exceed ~5% of tokens, grow C instead.
    C = max(P, -(-maxc // P) * P)  # round up to multiple of 128
    if maxc > 1024:
        overflow = sum(max(0, len(t) - 1024) for t in toks)
        if overflow <= 0.05 * sum(len(t) for t in toks):
            C = 1024
    host_toks = [t[C:] for t in toks]
    host_wts = [w[C:] for w in wts]
    toks = [t[:C] for t in toks]
    wts = [w[:C] for w in wts]

    if C not in _cache:
        _cache[C] = _build(C)
    nc = _cache[C]

    bf16 = ml_dtypes.bfloat16
    # weights: [E, F, H] -> [E, P, KH, F] etc. (transposed, 128-partition tiled)
    w1t = np.ascontiguousarray(
        np.asarray(w1, np.float32).reshape(E, F, KH, P).transpose(0, 3, 2, 1)
    ).astype(bf16)
    w3t = np.ascontiguousarray(
        np.asarray(w3, np.float32).reshape(E, F, KH, P).transpose(0, 3, 2, 1)
    ).astype(bf16)
    w2t = np.ascontiguousarray(
        np.asarray(w2, np.float32).reshape(E, H, KF, P).transpose(0, 3, 2, 1)
    ).astype(bf16)

    in_maps = []
    for e in range(E):
        xe = np.zeros((C, H), np.float32)
        xe[: len(toks[e])] = x[toks[e]]
        xt = np.ascontiguousarray(
            xe.reshape(C, KH, P).transpose(2, 1, 0)
        ).astype(bf16)
        in_maps.append({"xt": xt, "w1t": w1t[e], "w3t": w3t[e], "w2t": w2t[e]})

    trace = bool(int(os.environ.get("KERNEL_TRACE", "0")))
    if trace:
        try:
            from antenv.axon_hooks import get_axon_ntff_profile_hook  # noqa: F401
        except ImportError:
            trace = False  # profiling hook unavailable; run untraced
    res = run_bass_kernel_spmd(nc, in_maps, list(range(E)), trace=trace)
    kernel.last_results = res

    final = np.zeros((T, H), np.float32)
    for e in range(E):
        ne = len(toks[e])
        yt = res.results[e]["yt"]                     # [P, KH, C] f32
        y = yt.transpose(2, 1, 0).reshape(C, H)[:ne]  # [ne, H]
        final[toks[e]] += wts[e][:, None] * y
        if len(host_toks[e]):
            # overflow tokens: exact f32 FFN on host (~1% of the load)
            xo = x[host_toks[e]]
            h1 = xo @ np.asarray(w1[e], np.float32).T
            h3 = xo @ np.asarray(w3[e], np.float32).T
            gg = (h1 / (1.0 + np.exp(-h1))) * h3
            final[host_toks[e]] += host_wts[e][:, None] * (
                gg @ np.asarray(w2[e], np.float32).T)

    return final.reshape(b, s, h), logits


# revision 27
# speedup vs baseline: 1.0030x; 1.0030x over previous
"""MoE (Mixtral sparse block) Trainium2 kernel.

Strategy (expert-parallel, per sharding hint):
  - Host: compute router logits/softmax/top-2 in f32 (tiny: T x E x H),
    dispatch tokens to experts (this is the "all-to-all" -- done during
    host-side sharding, which the full-IO contract allows).
  - Device (8 cores, SPMD): core e runs expert e's FFN on its gathered
    tokens: y = (silu(x @ w1^T) * (x @ w3^T)) @ w2^T. bf16 operands,
    f32 PSUM accumulation, f32 output.
  - Host: combine = scatter-add rw-weighted expert outputs; return
    (final, router_logits) exactly like the reference.

Layouts: all device tensors are pre-transposed on host so the expert
weights are the stationary matmul operands and activations stream as the
moving operand; both FFN stages consume/produce [feature, token] layout,
so no on-device transposes are needed.
"""

import os
import numpy as np
import ml_dtypes

H = 1024
F = 2048
E = 8
TOPK = 2
P = 128
KH = H // P   # 8  contraction subtiles for H
KF = F // P   # 16 contraction subtiles for F
NBLK = 512    # moving-dim (token) block per matmul

_cache = {}


def _build(C):
    """Build the SPMD Bass program for per-core token capacity C."""
    import concourse.mybir as mybir
    from concourse import bacc
    from concourse.tile import TileContext

    dt = mybir.dt
    nc = bacc.Bacc()

    x_d = nc.declare_dram_parameter("xt", [P, KH, C], dt.bfloat16, isOutput=False)
    w1_d = nc.declare_dram_parameter("w1t", [P, KH, F], dt.bfloat16, isOutput=False)
    w3_d = nc.declare_dram_parameter("w3t", [P, KH, F], dt.bfloat16, isOutput=False)
    w2_d = nc.declare_dram_parameter("w2t", [P, KF, H], dt.bfloat16, isOutput=False)
    y_d = nc.declare_dram_parameter("yt", [P, KH, C], dt.float32, isOutput=True)

    cblocks = [(c0, min(NBLK, C - c0)) for c0 in range(0, C, NBLK)]
    NCB = len(cblocks)
    # w1/w3 DMA group sizes (in 128-col f tiles): small first so the first
    # matmuls wait on ~0.5MB, not 2MB.
    FGS = [1, 1, 2, 4, 4, 4]
    assert sum(FGS) == KF
    FG0 = [sum(FGS[:i]) for i in range(len(FGS))]  # start f-tile of group
    NFG = len(FGS)

    def fgroup(f):
        for g in range(NFG):
            if FG0[g] <= f < FG0[g] + FGS[g]:
                return g, f - FG0[g]
        raise AssertionError

    with TileContext(nc) as tc:
        with (
            tc.tile_pool(name="const", bufs=1) as cpool,
            tc.tile_pool(name="gated", bufs=2) as gpool,
            tc.tile_pool(name="tmp", bufs=3) as tpool,
            tc.tile_pool(name="outp", bufs=3) as opool,
            tc.tile_pool(name="psum", bufs=2, space="PSUM") as ppool,
        ):
            # PE warm-up: dummy matmuls on a zeroed tile from t=0 keep the
            # HAM activity window busy so the clock gate opens (1.2->2.4GHz)
            # before the first real matmuls are ready.
            zt = cpool.tile([P, NBLK], dt.bfloat16, tag="warm", name="warm")
            nc.vector.memset(zt[:], 0.0)
            pw = ppool.tile([P, NBLK], dt.float32, tag="wps", name="wps")
            for i in range(16):
                nc.tensor.matmul(pw[:], lhsT=zt[:, :P], rhs=zt[:],
                                 start=(i == 0), stop=(i == 15))

            # Split SBUF residents into small tiles so the first matmuls
            # only depend on the first ~1MB of DMA, not the full 15MB.
            xcb = [cpool.tile([P, KH, cb], dt.bfloat16, tag=f"x{j}", name=f"x{j}")
                   for j, (c0, cb) in enumerate(cblocks)]
            w1g = [cpool.tile([P, KH, FGS[g] * P], dt.bfloat16, tag=f"w1g{g}", name=f"w1g{g}")
                   for g in range(NFG)]
            w3g = [cpool.tile([P, KH, FGS[g] * P], dt.bfloat16, tag=f"w3g{g}", name=f"w3g{g}")
                   for g in range(NFG)]
            w2g = [cpool.tile([P, KF, P], dt.bfloat16, tag=f"w2g{h}", name=f"w2g{h}")
                   for h in range(KH)]

            # DMA in rough need-order. The first x block is split in two
            # halves so the f=0 half-groups can start ~2us earlier (deps are
            # range-granular).
            c00, cb0 = cblocks[0]
            hb = (cb0 // 2 // P) * P if cb0 >= 2 * P else cb0
            nc.sync.dma_start(xcb[0][:, :, :hb], x_d[:, :, c00:c00 + hb])
            nc.sync.dma_start(w1g[0][:], w1_d[:, :, FG0[0] * P:(FG0[0] + FGS[0]) * P])
            nc.sync.dma_start(w3g[0][:], w3_d[:, :, FG0[0] * P:(FG0[0] + FGS[0]) * P])
            if hb < cb0:
                nc.sync.dma_start(xcb[0][:, :, hb:cb0], x_d[:, :, c00 + hb:c00 + cb0])
            for g in range(1, NFG):
                nc.sync.dma_start(w1g[g][:], w1_d[:, :, FG0[g] * P:(FG0[g] + FGS[g]) * P])
                nc.sync.dma_start(w3g[g][:], w3_d[:, :, FG0[g] * P:(FG0[g] + FGS[g]) * P])
            for j in range(1, NCB):
                c0, cb = cblocks[j]
                nc.sync.dma_start(xcb[j][:], x_d[:, :, c0:c0 + cb])
            for h in range(KH):
                nc.sync.dma_start(w2g[h][:], w2_d[:, :, h * P:(h + 1) * P])

            all_gts = {}

            def emit_s1(j, f_lo, f_hi):
                c0, cb = cblocks[j]
                gts = all_gts.setdefault(j, [gpool.tile([P, cb], dt.bfloat16, tag=f"g{f}", name=f"g{j}_{f}")
                                             for f in range(KF)])
                for f in range(f_lo, f_hi):
                    g, fi = fgroup(f)
                    if j == 0 and f == 0 and cb >= 2 * P:
                        h0 = (cb // 2 // P) * P
                        spans = [(0, h0), (h0, cb - h0)]
                    else:
                        spans = [(0, cb)]
                    for o, w in spans:
                        ph1 = ppool.tile([P, NBLK], dt.float32, tag="h1")
                        ph3 = ppool.tile([P, NBLK], dt.float32, tag="h3")
                        for k in range(KH):
                            nc.tensor.matmul(
                                ph1[:, :w],
                                lhsT=w1g[g][:, k, fi * P:(fi + 1) * P],
                                rhs=xcb[j][:, k, o:o + w],
                                start=(k == 0),
                                stop=(k == KH - 1),
                            )
                        for k in range(KH):
                            nc.tensor.matmul(
                                ph3[:, :w],
                                lhsT=w3g[g][:, k, fi * P:(fi + 1) * P],
                                rhs=xcb[j][:, k, o:o + w],
                                start=(k == 0),
                                stop=(k == KH - 1),
                            )
                        st = tpool.tile([P, NBLK], dt.float32, tag="sig")
                        nc.scalar.activation(
                            st[:, :w], ph1[:, :w],
                            mybir.ActivationFunctionType.Sigmoid,
                        )
                        nc.vector.tensor_mul(st[:, :w], st[:, :w], ph1[:, :w])
                        nc.vector.tensor_mul(gts[f][:, o:o + w], st[:, :w], ph3[:, :w])

            def emit_s3(j):
                c0, cb = cblocks[j]
                gts = all_gts[j]
                for h in range(KH):
                    po = ppool.tile([P, NBLK], dt.float32, tag="out")
                    for k2 in range(KF):
                        nc.tensor.matmul(
                            po[:, :cb],
                            lhsT=w2g[h][:, k2, :],
                            rhs=gts[k2][:],
                            start=(k2 == 0),
                            stop=(k2 == KF - 1),
                        )
                    ot = opool.tile([P, NBLK], dt.float32, tag="ot")
                    nc.vector.tensor_copy(ot[:, :cb], po[:, :cb])
                    nc.sync.dma_start(y_d[:, h, c0:c0 + cb], ot[:, :cb])

            # Interleave: before each block's stage-3, start the next
            # block's first two stage-1 f-tiles so the PE has work while
            # the DVE finishes the current block's gated tiles.
            for j in range(NCB):
                emit_s1(j, 0 if j == 0 else 2, KF)
                if j + 1 < NCB:
                    emit_s1(j + 1, 0, 2)
                emit_s3(j)
    nc.finalize()
    return nc


def _route(x, gate_w):
    """f32 router matching the reference: softmax -> top-2 -> renormalize."""
    logits = x @ gate_w.T                                   # [T, E] f32
    m = logits.max(axis=-1, keepdims=True)
    p = np.exp((logits - m).astype(np.float32))
    p /= p.sum(axis=-1, keepdims=True)
    sel = np.argsort(-p, axis=-1, kind="stable")[:, :TOPK]  # [T, k]
    rw = np.take_along_axis(p, sel, axis=-1)
    rw = (rw / rw.sum(axis=-1, keepdims=True)).astype(np.float32)
    return logits, sel, rw


def kernel(hidden_states, gate_w, w1, w2, w3):
    from concourse.bass_utils import run_bass_kernel_spmd

    b, s, h = hidden_states.shape
    x = np.ascontiguousarray(hidden_states, dtype=np.float32).reshape(-1, h)
    T = x.shape[0]
    gate_w = np.asarray(gate_w, dtype=np.float32)

    logits, sel, rw = _route(x, gate_w)

    # token lists per expert
    toks, wts = [], []
    for e in range(E):
        mask = sel == e
        te = np.nonzero(mask.any(axis=-1))[0]
        toks.append(te)
        wts.append((rw * mask).sum(axis=-1)[te].astype(np.float32))
    maxc = max(len(t) for t in toks)
    # Cap device capacity at 1024 (two clean 512 blocks, zero padding);
    # the few overflow tokens of above-average experts (~1% of the load)
    # are folded in on the host during the combine. If overflow would
    # exceed ~5% of tokens, grow C instead.
    C = max(P, -(-maxc // P) * P)  # round up to multiple of 128
    if maxc > 1024:
        overflow = sum(max(0, len(t) - 1024) for t in toks)
        if overflow <= 0.05 * sum(len(t) for t in toks):
            C = 1024
    host_toks = [t[C:] for t in toks]
    host_wts = [w[C:] for w in wts]
    toks = [t[:C] for t in toks]
    wts = [w[:C] for w in wts]

    if C not in _cache:
        _cache[C] = _build(C)
    nc = _cache[C]

    bf16 = ml_dtypes.bfloat16
    # weights: [E, F, H] -> [E, P, KH, F] etc. (transposed, 128-partition tiled)
    w1t = np.ascontiguousarray(
        np.asarray(w1, np.float32).reshape(E, F, KH, P).transpose(0, 3, 2, 1)
    ).astype(bf16)
    w3t = np.ascontiguousarray(
        np.asarray(w3, np.float32).reshape(E, F, KH, P).transpose(0, 3, 2, 1)
    ).astype(bf16)
    w2t = np.ascontiguousarray(
        np.asarray(w2, np.float32).reshape(E, H, KF, P).transpose(0, 3, 2, 1)
    ).astype(bf16)

    in_maps = []
    for e in range(E):
        xe = np.zeros((C, H), np.float32)
        xe[: len(toks[e])] = x[toks[e]]
        xt = np.ascontiguousarray(
            xe.reshape(C, KH, P).transpose(2, 1, 0)
        ).astype(bf16)
        in_maps.append({"xt": xt, "w1t": w1t[e], "w3t": w3t[e], "w2t": w2t[e]})

    trace = bool(int(os.environ.get("KERNEL_TRACE", "0")))
    if trace:
        try:
            from antenv.axon_hooks import get_axon_ntff_profile_hook  # noqa: F401
        except ImportError:
            trace = False  # profiling hook unavailable; run untraced
    res = run_bass_kernel_spmd(nc, in_maps, list(range(E)), trace=trace)
    kernel.last_results = res

    final = np.zeros((T, H), np.float32)
    for e in range(E):
        ne = len(toks[e])
        yt = res.results[e]["yt"]                     # [P, KH, C] f32
        y = yt.transpose(2, 1, 0).reshape(C, H)[:ne]  # [ne, H]
        final[toks[e]] += wts[e][:, None] * y
        if len(host_toks[e]):
            # overflow tokens: exact f32 FFN on host (~1% of the load)
            xo = x[host_toks[e]]
            h1 = xo @ np.asarray(w1[e], np.float32).T
            h3 = xo @ np.asarray(w3[e], np.float32).T
            gg = (h1 / (1.0 + np.exp(-h1))) * h3
            final[host_toks[e]] += host_wts[e][:, None] * (
                gg @ np.asarray(w2[e], np.float32).T)

    return final.reshape(b, s, h), logits


# revision 28
# speedup vs baseline: 1.0102x; 1.0072x over previous
"""MoE (Mixtral sparse block) Trainium2 kernel.

Strategy (expert-parallel, per sharding hint):
  - Host: compute router logits/softmax/top-2 in f32 (tiny: T x E x H),
    dispatch tokens to experts (this is the "all-to-all" -- done during
    host-side sharding, which the full-IO contract allows).
  - Device (8 cores, SPMD): core e runs expert e's FFN on its gathered
    tokens: y = (silu(x @ w1^T) * (x @ w3^T)) @ w2^T. bf16 operands,
    f32 PSUM accumulation, f32 output.
  - Host: combine = scatter-add rw-weighted expert outputs; return
    (final, router_logits) exactly like the reference.

Layouts: all device tensors are pre-transposed on host so the expert
weights are the stationary matmul operands and activations stream as the
moving operand; both FFN stages consume/produce [feature, token] layout,
so no on-device transposes are needed.
"""

import os
import numpy as np
import ml_dtypes

H = 1024
F = 2048
E = 8
TOPK = 2
P = 128
KH = H // P   # 8  contraction subtiles for H
KF = F // P   # 16 contraction subtiles for F
NBLK = 512    # moving-dim (token) block per matmul

_cache = {}


def _build(C):
    """Build the SPMD Bass program for per-core token capacity C."""
    import concourse.mybir as mybir
    from concourse import bacc
    from concourse.tile import TileContext

    dt = mybir.dt
    nc = bacc.Bacc()

    x_d = nc.declare_dram_parameter("xt", [P, KH, C], dt.bfloat16, isOutput=False)
    w1_d = nc.declare_dram_parameter("w1t", [P, KH, F], dt.bfloat16, isOutput=False)
    w3_d = nc.declare_dram_parameter("w3t", [P, KH, F], dt.bfloat16, isOutput=False)
    w2_d = nc.declare_dram_parameter("w2t", [P, KF, H], dt.bfloat16, isOutput=False)
    y_d = nc.declare_dram_parameter("yt", [P, KH, C], dt.float32, isOutput=True)

    cblocks = [(c0, min(NBLK, C - c0)) for c0 in range(0, C, NBLK)]
    NCB = len(cblocks)
    # w1/w3 DMA group sizes (in 128-col f tiles): small first so the first
    # matmuls wait on ~0.5MB, not 2MB.
    FGS = [1, 1, 2, 4, 4, 4]
    assert sum(FGS) == KF
    FG0 = [sum(FGS[:i]) for i in range(len(FGS))]  # start f-tile of group
    NFG = len(FGS)

    def fgroup(f):
        for g in range(NFG):
            if FG0[g] <= f < FG0[g] + FGS[g]:
                return g, f - FG0[g]
        raise AssertionError

    with TileContext(nc) as tc:
        with (
            tc.tile_pool(name="const", bufs=1) as cpool,
            tc.tile_pool(name="gated", bufs=2) as gpool,
            tc.tile_pool(name="tmp", bufs=3) as tpool,
            tc.tile_pool(name="outp", bufs=3) as opool,
            tc.tile_pool(name="psum", bufs=2, space="PSUM") as ppool,
        ):
            # PE warm-up: dummy matmuls on a zeroed tile from t=0 keep the
            # HAM activity window busy so the clock gate opens (1.2->2.4GHz)
            # before the first real matmuls are ready.
            zt = cpool.tile([P, NBLK], dt.bfloat16, tag="warm", name="warm")
            nc.vector.memset(zt[:], 0.0)
            pw = ppool.tile([P, NBLK], dt.float32, tag="wps", name="wps")
            for i in range(16):
                nc.tensor.matmul(pw[:], lhsT=zt[:, :P], rhs=zt[:],
                                 start=(i == 0), stop=(i == 15))

            # Split SBUF residents into small tiles so the first matmuls
            # only depend on the first ~1MB of DMA, not the full 15MB.
            xcb = [cpool.tile([P, KH, cb], dt.bfloat16, tag=f"x{j}", name=f"x{j}")
                   for j, (c0, cb) in enumerate(cblocks)]
            w1g = [cpool.tile([P, KH, FGS[g] * P], dt.bfloat16, tag=f"w1g{g}", name=f"w1g{g}")
                   for g in range(NFG)]
            w3g = [cpool.tile([P, KH, FGS[g] * P], dt.bfloat16, tag=f"w3g{g}", name=f"w3g{g}")
                   for g in range(NFG)]
            w2g = [cpool.tile([P, KF, P], dt.bfloat16, tag=f"w2g{h}", name=f"w2g{h}")
                   for h in range(KH)]

            # DMA in rough need-order. The first x block is split in two
            # halves so the f=0 half-groups can start ~2us earlier (deps are
            # range-granular).
            c00, cb0 = cblocks[0]
            hb = (cb0 // 2 // P) * P if cb0 >= 2 * P else cb0
            nc.sync.dma_start(xcb[0][:, :, :hb], x_d[:, :, c00:c00 + hb])
            nc.sync.dma_start(w1g[0][:], w1_d[:, :, FG0[0] * P:(FG0[0] + FGS[0]) * P])
            nc.sync.dma_start(w3g[0][:], w3_d[:, :, FG0[0] * P:(FG0[0] + FGS[0]) * P])
            if hb < cb0:
                nc.sync.dma_start(xcb[0][:, :, hb:cb0], x_d[:, :, c00 + hb:c00 + cb0])
            for g in range(1, NFG):
                nc.sync.dma_start(w1g[g][:], w1_d[:, :, FG0[g] * P:(FG0[g] + FGS[g]) * P])
                nc.sync.dma_start(w3g[g][:], w3_d[:, :, FG0[g] * P:(FG0[g] + FGS[g]) * P])
            for j in range(1, NCB):
                c0, cb = cblocks[j]
                nc.sync.dma_start(xcb[j][:], x_d[:, :, c0:c0 + cb])
            for h in range(KH):
                nc.sync.dma_start(w2g[h][:], w2_d[:, :, h * P:(h + 1) * P])

            all_gts = {}

            def emit_s1(j, f_lo, f_hi):
                c0, cb = cblocks[j]
                gts = all_gts.setdefault(j, [gpool.tile([P, cb], dt.bfloat16, tag=f"g{f}", name=f"g{j}_{f}")
                                             for f in range(KF)])
                for f in range(f_lo, f_hi):
                    g, fi = fgroup(f)
                    if j == 0 and f == 0 and cb >= 2 * P:
                        h0 = (cb // 2 // P) * P
                        spans = [(0, h0), (h0, cb - h0)]
                    else:
                        spans = [(0, cb)]
                    for o, w in spans:
                        ph1 = ppool.tile([P, NBLK], dt.float32, tag="h1")
                        ph3 = ppool.tile([P, NBLK], dt.float32, tag="h3")
                        for k in range(KH):
                            nc.tensor.matmul(
                                ph1[:, :w],
                                lhsT=w1g[g][:, k, fi * P:(fi + 1) * P],
                                rhs=xcb[j][:, k, o:o + w],
                                start=(k == 0),
                                stop=(k == KH - 1),
                            )
                        for k in range(KH):
                            nc.tensor.matmul(
                                ph3[:, :w],
                                lhsT=w3g[g][:, k, fi * P:(fi + 1) * P],
                                rhs=xcb[j][:, k, o:o + w],
                                start=(k == 0),
                                stop=(k == KH - 1),
                            )
                        st = tpool.tile([P, NBLK], dt.float32, tag="sig")
                        nc.scalar.activation(
                            st[:, :w], ph1[:, :w],
                            mybir.ActivationFunctionType.Sigmoid,
                        )
                        nc.vector.tensor_mul(st[:, :w], st[:, :w], ph1[:, :w])
                        nc.vector.tensor_mul(gts[f][:, o:o + w], st[:, :w], ph3[:, :w])

            def emit_s3(j):
                c0, cb = cblocks[j]
                gts = all_gts[j]
                for h in range(KH):
                    # Last group of the whole kernel: split in half-width
                    # psum groups so the first half's copy+DMA overlaps the
                    # second half's matmuls (shorter serial tail).
                    if j == NCB - 1 and h == KH - 1 and cb >= 2 * P:
                        h0 = (cb // 2 // P) * P
                        spans = [(0, h0), (h0, cb - h0)]
                    else:
                        spans = [(0, cb)]
                    for o, w in spans:
                        po = ppool.tile([P, NBLK], dt.float32, tag="out")
                        for k2 in range(KF):
                            nc.tensor.matmul(
                                po[:, :w],
                                lhsT=w2g[h][:, k2, :],
                                rhs=gts[k2][:, o:o + w],
                                start=(k2 == 0),
                                stop=(k2 == KF - 1),
                            )
                        ot = opool.tile([P, NBLK], dt.float32, tag="ot")
                        nc.vector.tensor_copy(ot[:, :w], po[:, :w])
                        nc.sync.dma_start(y_d[:, h, c0 + o:c0 + o + w], ot[:, :w])

            # Interleave: before each block's stage-3, start the next
            # block's first two stage-1 f-tiles so the PE has work while
            # the DVE finishes the current block's gated tiles.
            for j in range(NCB):
                emit_s1(j, 0 if j == 0 else 2, KF)
                if j + 1 < NCB:
                    emit_s1(j + 1, 0, 2)
                emit_s3(j)
    nc.finalize()
    return nc


def _route(x, gate_w):
    """f32 router matching the reference: softmax -> top-2 -> renormalize."""
    logits = x @ gate_w.T                                   # [T, E] f32
    m = logits.max(axis=-1, keepdims=True)
    p = np.exp((logits - m).astype(np.float32))
    p /= p.sum(axis=-1, keepdims=True)
    sel = np.argsort(-p, axis=-1, kind="stable")[:, :TOPK]  # [T, k]
    rw = np.take_along_axis(p, sel, axis=-1)
    rw = (rw / rw.sum(axis=-1, keepdims=True)).astype(np.float32)
    return logits, sel, rw


def kernel(hidden_states, gate_w, w1, w2, w3):
    from concourse.bass_utils import run_bass_kernel_spmd

    b, s, h = hidden_states.shape
    x = np.ascontiguousarray(hidden_states, dtype=np.float32).reshape(-1, h)
    T = x.shape[0]
    gate_w = np.asarray(gate_w, dtype=np.float32)

    logits, sel, rw = _route(x, gate_w)

    # token lists per expert
    toks, wts = [], []
    for e in range(E):
        mask = sel == e
        te = np.nonzero(mask.any(axis=-1))[0]
        toks.append(te)
        wts.append((rw * mask).sum(axis=-1)[te].astype(np.float32))
    maxc = max(len(t) for t in toks)
    # Cap device capacity at 1024 (two clean 512 blocks, zero padding);
    # the few overflow tokens of above-average experts (~1% of the load)
    # are folded in on the host during the combine. If overflow would
    # exceed ~5% of tokens, grow C instead.
    C = max(P, -(-maxc // P) * P)  # round up to multiple of 128
    if maxc > 1024:
        overflow = sum(max(0, len(t) - 1024) for t in toks)
        if overflow <= 0.05 * sum(len(t) for t in toks):
            C = 1024
    host_toks = [t[C:] for t in toks]
    host_wts = [w[C:] for w in wts]
    toks = [t[:C] for t in toks]
    wts = [w[:C] for w in wts]

    if C not in _cache:
        _cache[C] = _build(C)
    nc = _cache[C]

    bf16 = ml_dtypes.bfloat16
    # weights: [E, F, H] -> [E, P, KH, F] etc. (transposed, 128-partition tiled)
    w1t = np.ascontiguousarray(
        np.asarray(w1, np.float32).reshape(E, F, KH, P).transpose(0, 3, 2, 1)
    ).astype(bf16)
    w3t = np.ascontiguousarray(
        np.asarray(w3, np.float32).reshape(E, F, KH, P).transpose(0, 3, 2, 1)
    ).astype(bf16)
    w2t = np.ascontiguousarray(
        np.asarray(w2, np.float32).reshape(E, H, KF, P).transpose(0, 3, 2, 1)
    ).astype(bf16)

    in_maps = []
    for e in range(E):
        xe = np.zeros((C, H), np.float32)
        xe[: len(toks[e])] = x[toks[e]]
        xt = np.ascontiguousarray(
            xe.reshape(C, KH, P).transpose(2, 1, 0)
        ).astype(bf16)
        in_maps.append({"xt": xt, "w1t": w1t[e], "w3t": w3t[e], "w2t": w2t[e]})

    trace = bool(int(os.environ.get("KERNEL_TRACE", "0")))
    if trace:
        try:
            from antenv.axon_hooks import get_axon_ntff_profile_hook  # noqa: F401
        except ImportError:
            trace = False  # profiling hook unavailable; run untraced
    res = run_bass_kernel_spmd(nc, in_maps, list(range(E)), trace=trace)
    kernel.last_results = res

    final = np.zeros((T, H), np.float32)
    for e in range(E):
        ne = len(toks[e])
        yt = res.results[e]["yt"]                     # [P, KH, C] f32
        y = yt.transpose(2, 1, 0).reshape(C, H)[:ne]  # [ne, H]
        final[toks[e]] += wts[e][:, None] * y
        if len(host_toks[e]):
            # overflow tokens: exact f32 FFN on host (~1% of the load)
            xo = x[host_toks[e]]
            h1 = xo @ np.asarray(w1[e], np.float32).T
            h3 = xo @ np.asarray(w3[e], np.float32).T
            gg = (h1 / (1.0 + np.exp(-h1))) * h3
            final[host_toks[e]] += host_wts[e][:, None] * (
                gg @ np.asarray(w2[e], np.float32).T)

    return final.reshape(b, s, h), logits
